# revision 29
# baseline (speedup 1.0000x reference)
"""AdaptivePConv Trainium2 kernel (8 NeuronCores, data-parallel over batch).

Per core (sample b = core index), on-device:
  1. per-channel min/max -> exact bins q = clip(trunc((x-mn)*recip(mx-mn+eps)*256), 0, 255)
     (recip is bit-identical to IEEE 1/d on this HW; trunc via magic-number
      round-to-nearest + compare fixup -> exact floor).  q stored as bf16
     (integers 0..255 are exact in bf16).
  2. per-channel 256-bin histogram via 255 fused threshold-count passes
     split across DVE (tensor_scalar is_ge in 4x bf16 mode), ACT (Sign +
     accum) and GPSIMD (tensor_scalar is_ge)
  3. entropy per channel (ACT Ln), top-16 channels via max8/match_replace
  4. selected channels -> 3x3 conv as 9-tap K=16 matmuls on PE
  5. unselected channels copied to output tail in ascending order via
     indirect-scatter DMA (selected rows skipped via bounds check)
"""
import os
import sys
import numpy as np

sys.path.insert(0, "/opt/trn_rl_repo")

import concourse.bass as bass
import concourse.bacc as bacc
import concourse.tile as tile
from concourse import mybir
import concourse.bass_utils as bu

F32 = mybir.dt.float32
BF16 = mybir.dt.bfloat16
I32 = mybir.dt.int32
U32 = mybir.dt.uint32
Alu = mybir.AluOpType
Act = mybir.ActivationFunctionType
AxX = mybir.AxisListType.X

B, C, H, W = 8, 64, 256, 256
N = H * W                  # 65536 pixels per channel
NHALF = N // 2             # 32768 per (channel, half) partition
OC, P_SEL = 64, 16
C_OUT = OC + C - P_SEL     # 112
NBINS = 256
# threshold split across engines (k=0..254; cge[k] = count(q >= k+1))
# (GPSIMD/Pool cannot run generic tensor ops on NC-v3 -- DVE + ACT only)
ND = 195                   # DVE thresholds (4x bf16 tensor_scalar)
NA = 60                    # ACT thresholds (Sign + accum)
FQ = 8192                  # free-size per histogram instruction
NQI = NHALF // FQ          # 4 sub-accumulators per threshold
MAGIC = float(np.float32(2.0 ** 23))
XCH = 1024                 # prepass chunk
NCH = NHALF // XCH         # 32 chunks


def bcast(ap_small, ap_big):
    return bass.broadcast_tensor_aps(ap_small, ap_big)[0]


def build():
    nc = bacc.Bacc()
    x_ext = nc.declare_dram_parameter("x", [C, N], F32, isOutput=False)
    w_ext = nc.declare_dram_parameter("w", [16, 9 * OC], F32, isOutput=False)
    bias_ext = nc.declare_dram_parameter("bias", [OC, 1], F32, isOutput=False)
    biasA_ext = nc.declare_dram_parameter("biasA", [128, 255], F32, isOutput=False)
    blk_ext = nc.declare_dram_parameter("blkvec", [128, 1], F32, isOutput=False)
    iota_ext = nc.declare_dram_parameter("colio", [1, 64], F32, isOutput=False)
    cid_ext = nc.declare_dram_parameter("cid64", [64, 1], F32, isOutput=False)
    out_ext = nc.declare_dram_parameter("out", [C_OUT, N], F32, isOutput=True)
    dbg_act = nc.declare_dram_parameter("dbg_act", [C, 1], F32, isOutput=True)
    dbg_idx = nc.declare_dram_parameter("dbg_idx", [1, 16], U32, isOutput=True)
    dbg_n = nc.declare_dram_parameter("dbg_n", [C, NBINS], F32, isOutput=True)

    scr_stat = nc.dram_tensor("scr_stat", [128], F32)
    scr_stat2 = nc.dram_tensor("scr_stat2", [64], F32)
    scr_cge = nc.dram_tensor("scr_cge", [128, 255], F32)
    scr_act = nc.dram_tensor("scr_act", [64], F32)
    scr_idx = nc.dram_tensor("scr_idx", [16], F32)
    scr_slot = nc.dram_tensor("scr_slot", [64], F32)
    scr_unsel = nc.dram_tensor("scr_unsel", [128], F32)

    def dram_ap(t, offset, pattern):
        return bass.AP(t, offset, pattern)

    with tile.TileContext(nc) as tc:
        with tc.tile_pool(name="persist", bufs=1) as pp:
            # ---- persistent small tiles ----
            wsb = pp.tile([16, 9 * OC], BF16)
            with tc.tile_pool(name="wtmp", bufs=1) as wt:
                wsbf = wt.tile([16, 9 * OC], F32)
                nc.sync.dma_start(wsbf[:], w_ext[:])
                nc.vector.tensor_copy(wsb[:], wsbf[:])
            biasC = pp.tile([OC, 1], F32)
            nc.sync.dma_start(biasC[:], bias_ext[:])
            biasA = pp.tile([128, 255], F32)
            nc.sync.dma_start(biasA[:], biasA_ext[:])
            colio = pp.tile([1, 64], F32)
            nc.sync.dma_start(colio[:], iota_ext[:])
            epsT = pp.tile([128, 1], F32)
            nc.vector.memset(epsT[:], 1e-8)
            zb = pp.tile([128, 1], F32)
            nc.vector.memset(zb[:], 0.0)
            z64 = pp.tile([1, 64], F32)
            nc.vector.memset(z64[:], 0.0)

            mn128 = pp.tile([128, 1], F32)
            r256 = pp.tile([128, 1], F32)
            accD = pp.tile([128, NQI * ND], F32)
            accA = pp.tile([128, NQI * NA], F32)

            # preallocate tree-sum scratch BEFORE qpool opens: allocating these
            # after qpool closes would alias the freed q/junk SBUF and put a
            # WAR-on-every-hist-instruction dependency on the critical path
            def make_tree_bufs(width):
                bufs, n_sub = {}, NQI
                while n_sub > 1:
                    half = n_sub // 2
                    bufs[half] = pp.tile([128, width * half], F32,
                                         name=f"ts{width}_{half}", tag=f"ts{width}_{half}")
                    n_sub = half
                return bufs
            tsD_bufs = make_tree_bufs(ND)
            cge = pp.tile([128, 255], F32)
            junkS = pp.tile([128, NQI], F32)
            biasH = pp.tile([128, 1], F32)
            nc.vector.memset(biasH[:], float(FQ // 2))

            with tc.tile_pool(name="qpool", bufs=1) as qp:
                q_sb = qp.tile([128, NHALF], BF16)
                junkD = qp.tile([128, FQ], BF16)
                junkA = qp.tile([128, FQ], BF16)

                # ============ phase 1: min/max ============
                with nc.named_scope("minmax"):
                    with tc.tile_pool(name="xminmax", bufs=3) as xp:
                        mxP = pp.tile([128, NCH], F32)
                        mnP = pp.tile([128, NCH], F32)
                        for h in range(NCH):
                            xh = xp.tile([128, XCH], F32, tag="xh")
                            nc.sync.dma_start(
                                xh[:],
                                x_ext[:].rearrange("c (t m) -> (c t) m", t=2)[:, h * XCH:(h + 1) * XCH])
                            nc.vector.tensor_reduce(mxP[:, h:h + 1], xh[:], axis=AxX, op=Alu.max)
                            nc.vector.tensor_reduce(mnP[:, h:h + 1], xh[:], axis=AxX, op=Alu.min)
                        mx1 = pp.tile([128, 1], F32)
                        mn1 = pp.tile([128, 1], F32)
                        nc.vector.tensor_reduce(mx1[:], mxP[:], axis=AxX, op=Alu.max)
                        nc.vector.tensor_reduce(mn1[:], mnP[:], axis=AxX, op=Alu.min)

                        # pair-combine (c,half) partitions via DRAM bounce, expand back
                        mx128 = pp.tile([128, 1], F32)
                        for src1, dst, op in ((mx1, mx128, Alu.max), (mn1, mn128, Alu.min)):
                            nc.sync.dma_start(dram_ap(scr_stat, 0, [[1, 128]]), src1[:])
                            ev = pp.tile([64, 1], F32, tag="ev")
                            od = pp.tile([64, 1], F32, tag="od")
                            nc.sync.dma_start(ev[:], dram_ap(scr_stat, 0, [[2, 64], [1, 1]]))
                            nc.sync.dma_start(od[:], dram_ap(scr_stat, 1, [[2, 64], [1, 1]]))
                            cmb = pp.tile([64, 1], F32, tag="cmb")
                            nc.vector.tensor_tensor(cmb[:], ev[:], od[:], op=op)
                            nc.sync.dma_start(dram_ap(scr_stat2, 0, [[1, 64]]), cmb[:])
                            nc.sync.dma_start(dst[:], dram_ap(scr_stat2, 0, [[1, 64], [0, 2]]))

                        dT = pp.tile([128, 1], F32)
                        nc.vector.scalar_tensor_tensor(dT[:], mx128[:], mn128[:], epsT[:],
                                                       op0=Alu.subtract, op1=Alu.add)
                        rT = pp.tile([128, 1], F32)
                        nc.vector.reciprocal(rT[:], dT[:])
                        nc.vector.tensor_scalar(r256[:], rT[:], 256.0, None, op0=Alu.mult)

                # ============ phase 2: q (exact trunc bins -> bf16) ============
                with nc.named_scope("qcomp"):
                    with tc.tile_pool(name="xq", bufs=2) as xp2:
                        for h in range(NCH):
                            eng = nc.vector
                            sl = slice(h * XCH, (h + 1) * XCH)
                            xh = xp2.tile([128, XCH], F32, tag="xh2")
                            nc.sync.dma_start(
                                xh[:],
                                x_ext[:].rearrange("c (t m) -> (c t) m", t=2)[:, sl])
                            t_ = xp2.tile([128, XCH], F32, tag="t_")
                            eng.scalar_tensor_tensor(t_[:], xh[:], mn128[:], bcast(r256[:], xh[:]),
                                                     op0=Alu.subtract, op1=Alu.mult)
                            y_ = xp2.tile([128, XCH], F32, tag="y_")
                            eng.tensor_scalar(y_[:], t_[:], MAGIC, None, op0=Alu.add)
                            eng.tensor_scalar(y_[:], y_[:], MAGIC, None, op0=Alu.subtract)
                            g_ = xp2.tile([128, XCH], F32, tag="g_")
                            eng.tensor_tensor(g_[:], y_[:], t_[:], op=Alu.is_gt)
                            eng.tensor_tensor(y_[:], y_[:], g_[:], op=Alu.subtract)
                            eng.tensor_scalar(q_sb[:, sl], y_[:], 255.0, None, op0=Alu.min)

                # ============ phase 3: histogram (threshold counts) ============
                def emit_D(i):
                    k, h2 = divmod(i, NQI)
                    nc.vector.tensor_scalar(
                        junkD[:], q_sb[:, h2 * FQ:(h2 + 1) * FQ], float(k + 1), None,
                        op0=Alu.is_ge, op1=Alu.add,
                        accum_out=accD[:, i:i + 1])

                def emit_A(i):
                    j, h2 = divmod(i, NQI)
                    nc.scalar.activation(
                        junkA[:], q_sb[:, h2 * FQ:(h2 + 1) * FQ], Act.Sign,
                        bias=biasA[:, ND + j:ND + j + 1], scale=1.0,
                        accum_out=accA[:, i:i + 1])

                # interleave emission across engines proportionally to their
                # instruction counts so periodic cross-engine sync points land
                # at similar progress on every engine
                with nc.named_scope("hist"):
                    streams = [(emit_D, ND * NQI), (emit_A, NA * NQI)]
                    done = [0, 0]
                    total = sum(n for _, n in streams)
                    for _ in range(total):
                        si = min(range(2), key=lambda s: done[s] / streams[s][1])
                        streams[si][0](done[si])
                        done[si] += 1

            # ============ phase 4: counts + entropy ============
            with nc.named_scope("post"):
                # sum NQI sub-accumulators per threshold. DVE: pairwise tree on
                # its own accD. ACT: one activation-accum per threshold — sums
                # (0.5*x + FQ/2) over its NQI sub-accs, which IS the count
                # (each ACT sub-acc is a sum of +-1 over FQ elements). Keeping
                # each engine's reduction on that engine avoids cross-engine
                # head-of-line blocking in the in-order streams.
                def tree_sum(eng, acc_tile, width, bufs):
                    cur_tile, n_sub = acc_tile, NQI
                    while n_sub > 1:
                        half = n_sub // 2
                        nxt = bufs[half]
                        nv = nxt[:].rearrange("p (k h) -> p k h", h=half)
                        cur = cur_tile[:].rearrange("p (k h) -> p k h", h=n_sub)
                        for i2 in range(half):
                            eng.tensor_tensor(nv[:, :, i2:i2 + 1],
                                              cur[:, :, 2 * i2:2 * i2 + 1],
                                              cur[:, :, 2 * i2 + 1:2 * i2 + 2], op=Alu.add)
                        cur_tile, n_sub = nxt, half
                    return cur_tile
                sD = tree_sum(nc.vector, accD, ND, tsD_bufs)
                nc.vector.tensor_copy(cge[:, 0:ND], sD[:])
                for j in range(NA):
                    nc.scalar.activation(
                        junkS[:], accA[:, NQI * j:NQI * (j + 1)], Act.Copy,
                        bias=float(FQ // 2), scale=0.5,
                        accum_out=cge[:, ND + j:ND + j + 1])
                # combine (c, half) partition pairs -> [64, 255]
                nc.sync.dma_start(scr_cge[:], cge[:])
                cgeE = pp.tile([64, 255], F32)
                cgeO = pp.tile([64, 255], F32)
                nc.sync.dma_start(cgeE[:], dram_ap(scr_cge, 0, [[510, 64], [1, 255]]))
                nc.sync.dma_start(cgeO[:], dram_ap(scr_cge, 255, [[510, 64], [1, 255]]))
                cgeC = pp.tile([64, 255], F32)
                nc.vector.tensor_tensor(cgeC[:], cgeE[:], cgeO[:], op=Alu.add)

                nT = pp.tile([64, NBINS], F32)
                nc.vector.tensor_scalar(nT[:, 0:1], cgeC[:, 0:1], -1.0, float(N), op0=Alu.mult, op1=Alu.add)
                nc.vector.tensor_tensor(nT[:, 1:255], cgeC[:, 0:254], cgeC[:, 1:255], op=Alu.subtract)
                nc.vector.tensor_copy(nT[:, 255:256], cgeC[:, 254:255])
                nc.sync.dma_start(dbg_n[:], nT[:])

                histT = pp.tile([64, NBINS], F32)
                nc.vector.tensor_scalar(histT[:], nT[:], 1e-8, None, op0=Alu.add)
                S_ = pp.tile([64, 1], F32)
                nc.vector.tensor_reduce(S_[:], histT[:], axis=AxX, op=Alu.add)
                rS = pp.tile([64, 1], F32)
                nc.vector.reciprocal(rS[:], S_[:])
                probT = pp.tile([64, NBINS], F32)
                nc.vector.tensor_scalar(probT[:], histT[:], rS[:], None, op0=Alu.mult)
                pe_ = pp.tile([64, NBINS], F32)
                nc.vector.tensor_scalar(pe_[:], probT[:], 1e-8, None, op0=Alu.add)
                lnT = pp.tile([64, NBINS], F32)
                nc.scalar.activation(lnT[:], pe_[:], Act.Ln, bias=zb[0:64, :], scale=1.0)
                termT = pp.tile([64, NBINS], F32)
                nc.vector.tensor_tensor(termT[:], probT[:], lnT[:], op=Alu.mult)
                actT = pp.tile([64, 1], F32)
                nc.vector.tensor_reduce(actT[:], termT[:], axis=AxX, op=Alu.add, negate=True)
                nc.sync.dma_start(dbg_act[:], actT[:])

                # ============ phase 5: top-16 ============
                nc.sync.dma_start(dram_ap(scr_act, 0, [[1, 64]]), actT[:])
                act64 = pp.tile([1, 64], F32)
                nc.sync.dma_start(act64[:], dram_ap(scr_act, 0, [[64, 1], [1, 64]]))
                idx16 = pp.tile([1, 16], U32)
                m8 = pp.tile([1, 8], F32)
                nc.vector.max(m8[:], act64[:])
                nc.vector.max_index(idx16[:, 0:8], m8[:], act64[:])
                act64b = pp.tile([1, 64], F32)
                nc.vector.match_replace(act64b[:], m8[:], act64[:], -3.0e38)
                m8b = pp.tile([1, 8], F32)
                nc.vector.max(m8b[:], act64b[:])
                nc.vector.max_index(idx16[:, 8:16], m8b[:], act64b[:])
                act64c = pp.tile([1, 64], F32)
                nc.vector.match_replace(act64c[:], m8b[:], act64b[:], -3.0e38)
                nc.sync.dma_start(dbg_idx[:], idx16[:])

                # ============ phase 6: selection metadata ============
                idx16f = pp.tile([1, 16], F32)
                nc.vector.tensor_copy(idx16f[:], idx16[:])
                nc.sync.dma_start(dram_ap(scr_idx, 0, [[1, 16]]), idx16f[:])
                idx16T = pp.tile([16, 1], F32)
                nc.sync.dma_start(idx16T[:], dram_ap(scr_idx, 0, [[1, 16], [16, 1]]))
                goffF8 = pp.tile([16, 16], F32)
                for j in range(16):
                    nc.vector.tensor_scalar(goffF8[:, j:j + 1], idx16T[:], 16.0, float(j),
                                            op0=Alu.mult, op1=Alu.add)
                goffI8 = pp.tile([16, 16], I32)
                nc.vector.tensor_copy(goffI8[:], goffF8[:])

                mask01 = pp.tile([1, 64], F32)
                nc.vector.tensor_scalar(mask01[:], act64c[:], -1.0e38, None, op0=Alu.is_le)
                rank = pp.tile([1, 64], F32)
                nc.vector.tensor_tensor_scan(rank[:], mask01[:], z64[:], 0.0, op0=Alu.add, op1=Alu.add)
                sl1 = pp.tile([1, 64], F32)
                nc.vector.tensor_tensor(sl1[:], colio[:], rank[:], op=Alu.subtract)
                sl2 = pp.tile([1, 64], F32)
                nc.vector.tensor_scalar(sl2[:], sl1[:], 64.0, None, op0=Alu.add)
                slotf = pp.tile([1, 64], F32)
                nc.vector.scalar_tensor_tensor(slotf[:], mask01[:], 100000.0, sl2[:],
                                               op0=Alu.mult, op1=Alu.add)
                slot8 = pp.tile([1, 64], F32)
                nc.vector.tensor_scalar(slot8[:], slotf[:], 16.0, None, op0=Alu.mult)
                nc.sync.dma_start(dram_ap(scr_slot, 0, [[1, 64]]), slot8[:])
                slot8T = pp.tile([64, 1], F32)
                nc.sync.dma_start(slot8T[:], dram_ap(scr_slot, 0, [[1, 64], [64, 1]]))
                offsF = pp.tile([64, 16], F32)
                for j in range(16):
                    nc.vector.tensor_scalar(offsF[:, j:j + 1], slot8T[:], float(j), None, op0=Alu.add)
                offsI = pp.tile([64, 16], I32)
                nc.vector.tensor_copy(offsI[:], offsF[:])

            # ============ phase 7: selected gather + conv ============
            with nc.named_scope("conv"):
                with tc.tile_pool(name="convp", bufs=1) as cp:
                    selb = cp.tile([16, N], BF16)
                    GCH = N // 16
                    with tc.tile_pool(name="gathp", bufs=1) as gp:
                        for j in range(16):
                            ga = gp.tile([16, GCH], F32, tag="ga")
                            nc.gpsimd.indirect_dma_start(
                                out=ga[:], out_offset=None,
                                in_=x_ext[:].rearrange("c (t m) -> (c t) m", t=16),
                                in_offset=bass.IndirectOffsetOnAxis(ap=goffI8[:, j:j + 1], axis=0))
                            nc.vector.tensor_copy(selb[:, j * GCH:(j + 1) * GCH], ga[:])

                    with tc.tile_pool(name="psum", bufs=4, space="PSUM") as psp, \
                         tc.tile_pool(name="stage", bufs=2) as stp:
                        RPS = 8
                        for blk in range(32):
                            stage = stp.tile([OC, RPS * W], F32, tag="stage")
                            for yy in range(RPS):
                                y = blk * RPS + yy
                                ps = psp.tile([OC, W], F32, tag="ps")
                                taps = []
                                for dy in (-1, 0, 1):
                                    ys = y + dy
                                    if 0 <= ys < H:
                                        for dx in (-1, 0, 1):
                                            taps.append((dy, dx, ys))
                                # full-width (dx==0) tap first: the start=True
                                # matmul must zero the whole PSUM region
                                taps.sort(key=lambda t: t[1] != 0)
                                for ti, (dy, dx, ys) in enumerate(taps):
                                    t_idx = (dy + 1) * 3 + (dx + 1)
                                    if dx == -1:
                                        rhs = selb[:, ys * W:ys * W + (W - 1)]
                                        outp = ps[:, 1:W]
                                    elif dx == 1:
                                        rhs = selb[:, ys * W + 1:ys * W + W]
                                        outp = ps[:, 0:W - 1]
                                    else:
                                        rhs = selb[:, ys * W:ys * W + W]
                                        outp = ps[:, 0:W]
                                    nc.tensor.matmul(outp, wsb[:, t_idx * OC:(t_idx + 1) * OC], rhs,
                                                     start=(ti == 0), stop=(ti == len(taps) - 1))
                                nc.scalar.activation(stage[:, yy * W:(yy + 1) * W], ps[:],
                                                     Act.Identity, bias=biasC[:], scale=1.0)
                            nc.sync.dma_start(
                                out_ext[0:OC, blk * RPS * W:(blk + 1) * RPS * W], stage[:])

            # ============ phase 8: unselected passthrough ============
            # invert the channel->row map via a tiny indirect scatter of
            # channel ids into row order, then gather the 48 unselected
            # channels into SBUF and write the output tail with direct
            # contiguous DMAs (no large indirect scatter).
            with nc.named_scope("upass"):
                with tc.tile_pool(name="upmeta", bufs=1) as upm:
                    rowF = upm.tile([64, 1], F32)
                    nc.vector.tensor_scalar(rowF[:], slot8T[:], 1.0 / 16.0, None, op0=Alu.mult)
                    rowI = upm.tile([64, 1], I32)
                    nc.vector.tensor_copy(rowI[:], rowF[:])
                    cidT = upm.tile([64, 1], F32)
                    nc.sync.dma_start(cidT[:], cid_ext[:])
                    nc.gpsimd.indirect_dma_start(
                        out=dram_ap(scr_unsel, 0, [[1, 128], [1, 1]]),
                        out_offset=bass.IndirectOffsetOnAxis(ap=rowI[:], axis=0),
                        in_=cidT[:], in_offset=None,
                        bounds_check=127, oob_is_err=False)
                    unsel48 = upm.tile([48, 1], F32)
                    nc.sync.dma_start(unsel48[:], dram_ap(scr_unsel, 64, [[1, 48], [1, 1]]))
                    offs48F = upm.tile([48, 16], F32)
                    for j in range(16):
                        nc.vector.tensor_scalar(offs48F[:, j:j + 1], unsel48[:], 16.0, float(j),
                                                op0=Alu.mult, op1=Alu.add)
                    offs48I = upm.tile([48, 16], I32)
                    nc.vector.tensor_copy(offs48I[:], offs48F[:])
                    UCH = N // 16
                    with tc.tile_pool(name="upass", bufs=3) as up:
                        for j in range(16):
                            g = up.tile([48, UCH], F32, tag="ug")
                            nc.gpsimd.indirect_dma_start(
                                out=g[:], out_offset=None,
                                in_=x_ext[:].rearrange("c (t m) -> (c t) m", t=16),
                                in_offset=bass.IndirectOffsetOnAxis(ap=offs48I[:, j:j + 1], axis=0))
                            nc.sync.dma_start(out_ext[64:C_OUT, j * UCH:(j + 1) * UCH], g[:])
    nc.compile()
    return nc


_CACHED = {}


def _get_nc():
    if "nc" not in _CACHED:
        _CACHED["nc"] = build()
    return _CACHED["nc"]


def make_inputs_per_core(x, weight, bias):
    x = np.ascontiguousarray(x, dtype=np.float32)
    weight = np.asarray(weight, dtype=np.float32)
    bias = np.asarray(bias, dtype=np.float32)
    wt = np.ascontiguousarray(np.transpose(weight, (1, 2, 3, 0)).reshape(16, 9 * OC))
    biasT = np.ascontiguousarray(bias.reshape(OC, 1))
    biasA = np.ascontiguousarray(
        np.broadcast_to(-(np.arange(255, dtype=np.float32) + 0.5), (128, 255)))
    blkvec = np.ascontiguousarray((np.arange(128, dtype=np.float32) // 16).reshape(128, 1))
    colio = np.ascontiguousarray(np.arange(64, dtype=np.float32).reshape(1, 64))
    cid64 = np.ascontiguousarray(np.arange(64, dtype=np.float32).reshape(64, 1))
    maps = []
    for b in range(B):
        maps.append({
            "x": np.ascontiguousarray(x[b].reshape(C, N)),
            "w": wt, "bias": biasT, "biasA": biasA,
            "blkvec": blkvec, "colio": colio, "cid64": cid64,
        })
    return maps


LAST_RESULT = {}


def kernel(x, weight, bias):
    nc = _get_nc()
    maps = make_inputs_per_core(x, weight, bias)
    trace = bool(int(os.environ.get("KERNEL_TRACE", "0")))
    if trace:
        sys.path.insert(0, os.path.dirname(os.path.abspath(__file__)))
        try:
            import profhook
            profhook.install()
        except Exception:
            trace = False
    res = bu.run_bass_kernel_spmd(nc, maps, list(range(8)), trace=trace)
    LAST_RESULT["res"] = res
    out = np.stack([res.results[i]["out"].reshape(C_OUT, H, W) for i in range(B)])
    return out


if __name__ == "__main__":
    import reference as R
    inputs = R.setup_inputs()
    out = kernel(np.asarray(inputs["x"]), np.asarray(inputs["weight"]),
                 np.asarray(inputs["bias"]))
    print("out shape:", out.shape)


# revision 30
# speedup vs baseline: 1.5075x; 1.5075x over previous
"""AdaptivePConv Trainium2 kernel (8 NeuronCores, data-parallel over batch).

Per core (sample b = core index), on-device:
  1. per-channel min/max -> exact bins q = clip(trunc((x-mn)*recip(mx-mn+eps)*256), 0, 255)
     (recip is bit-identical to IEEE 1/d on this HW; trunc via magic-number
      round-to-nearest + compare fixup -> exact floor).  q stored as bf16
     (integers 0..255 are exact in bf16).
  2. per-channel 256-bin histogram via 255 fused threshold-count passes
     split across DVE (tensor_scalar is_ge in 4x bf16 mode), ACT (Sign +
     accum) and GPSIMD (tensor_scalar is_ge)
  3. entropy per channel (ACT Ln), top-16 channels via max8/match_replace
  4. selected channels -> 3x3 conv as 9-tap K=16 matmuls on PE
  5. unselected channels copied to output tail in ascending order via
     indirect-scatter DMA (selected rows skipped via bounds check)
"""
import os
import sys
import numpy as np

sys.path.insert(0, "/opt/trn_rl_repo")

import concourse.bass as bass
import concourse.bacc as bacc
import concourse.tile as tile
from concourse import mybir
import concourse.bass_utils as bu

F32 = mybir.dt.float32
BF16 = mybir.dt.bfloat16
I32 = mybir.dt.int32
U32 = mybir.dt.uint32
Alu = mybir.AluOpType
Act = mybir.ActivationFunctionType
AxX = mybir.AxisListType.X

B, C, H, W = 8, 64, 256, 256
N = H * W                  # 65536 pixels per channel
NHALF = N // 2             # 32768 per (channel, half) partition
OC, P_SEL = 64, 16
C_OUT = OC + C - P_SEL     # 112
NBINS = 256
# threshold split across engines (k=0..254; cge[k] = count(q >= k+1))
# (GPSIMD/Pool cannot run generic tensor ops on NC-v3 -- DVE + ACT only)
ND = 114                   # DVE thresholds (HW: ~1.04ns/elem, accum kills fast modes)
NA = 141                   # ACT thresholds (HW: ~0.84ns/elem incl accum)
FQ = 8192                  # free-size per histogram instruction
NQI = NHALF // FQ          # 4 sub-accumulators per threshold
MAGIC = float(np.float32(2.0 ** 23))
XCH = 1024                 # prepass chunk
NCH = NHALF // XCH         # 32 chunks


def bcast(ap_small, ap_big):
    return bass.broadcast_tensor_aps(ap_small, ap_big)[0]


def build():
    nc = bacc.Bacc()
    x_ext = nc.declare_dram_parameter("x", [C, N], F32, isOutput=False)
    w_ext = nc.declare_dram_parameter("w", [16, 9 * OC], F32, isOutput=False)
    bias_ext = nc.declare_dram_parameter("bias", [OC, 1], F32, isOutput=False)
    biasA_ext = nc.declare_dram_parameter("biasA", [128, 255], F32, isOutput=False)
    blk_ext = nc.declare_dram_parameter("blkvec", [128, 1], F32, isOutput=False)
    iota_ext = nc.declare_dram_parameter("colio", [1, 64], F32, isOutput=False)
    cid_ext = nc.declare_dram_parameter("cid64", [64, 1], F32, isOutput=False)
    out_ext = nc.declare_dram_parameter("out", [C_OUT, N], F32, isOutput=True)
    dbg_act = nc.declare_dram_parameter("dbg_act", [C, 1], F32, isOutput=True)
    dbg_idx = nc.declare_dram_parameter("dbg_idx", [1, 16], U32, isOutput=True)
    dbg_n = nc.declare_dram_parameter("dbg_n", [C, NBINS], F32, isOutput=True)

    scr_stat = nc.dram_tensor("scr_stat", [128], F32)
    scr_stat2 = nc.dram_tensor("scr_stat2", [64], F32)
    scr_cge = nc.dram_tensor("scr_cge", [128, 255], F32)
    scr_act = nc.dram_tensor("scr_act", [64], F32)
    scr_idx = nc.dram_tensor("scr_idx", [16], F32)
    scr_slot = nc.dram_tensor("scr_slot", [64], F32)
    scr_unsel = nc.dram_tensor("scr_unsel", [128], F32)

    def dram_ap(t, offset, pattern):
        return bass.AP(t, offset, pattern)

    with tile.TileContext(nc) as tc:
        with tc.tile_pool(name="persist", bufs=1) as pp:
            # ---- persistent small tiles ----
            wsb = pp.tile([16, 9 * OC], BF16)
            with tc.tile_pool(name="wtmp", bufs=1) as wt:
                wsbf = wt.tile([16, 9 * OC], F32)
                nc.sync.dma_start(wsbf[:], w_ext[:])
                nc.vector.tensor_copy(wsb[:], wsbf[:])
            biasC = pp.tile([OC, 1], F32)
            nc.sync.dma_start(biasC[:], bias_ext[:])
            biasA = pp.tile([128, 255], F32)
            nc.sync.dma_start(biasA[:], biasA_ext[:])
            colio = pp.tile([1, 64], F32)
            nc.sync.dma_start(colio[:], iota_ext[:])
            epsT = pp.tile([128, 1], F32)
            nc.vector.memset(epsT[:], 1e-8)
            zb = pp.tile([128, 1], F32)
            nc.vector.memset(zb[:], 0.0)
            z64 = pp.tile([1, 64], F32)
            nc.vector.memset(z64[:], 0.0)

            mn128 = pp.tile([128, 1], F32)
            r256 = pp.tile([128, 1], F32)
            accD = pp.tile([128, NQI * ND], F32)
            accA = pp.tile([128, NQI * NA], F32)

            # preallocate tree-sum scratch BEFORE qpool opens: allocating these
            # after qpool closes would alias the freed q/junk SBUF and put a
            # WAR-on-every-hist-instruction dependency on the critical path
            def make_tree_bufs(width):
                bufs, n_sub = {}, NQI
                while n_sub > 1:
                    half = n_sub // 2
                    bufs[half] = pp.tile([128, width * half], F32,
                                         name=f"ts{width}_{half}", tag=f"ts{width}_{half}")
                    n_sub = half
                return bufs
            tsD_bufs = make_tree_bufs(ND)
            cge = pp.tile([128, 255], F32)
            junkS = pp.tile([128, NQI], F32)
            biasH = pp.tile([128, 1], F32)
            nc.vector.memset(biasH[:], float(FQ // 2))

            with tc.tile_pool(name="qpool", bufs=1) as qp:
                q_sb = qp.tile([128, NHALF], BF16)
                junkD = qp.tile([128, FQ], BF16)
                junkA = qp.tile([128, FQ], BF16)

                # ============ phase 1: min/max ============
                with nc.named_scope("minmax"):
                    with tc.tile_pool(name="xminmax", bufs=3) as xp:
                        mxP = pp.tile([128, NCH], F32)
                        mnP = pp.tile([128, NCH], F32)
                        for h in range(NCH):
                            xh = xp.tile([128, XCH], F32, tag="xh")
                            nc.sync.dma_start(
                                xh[:],
                                x_ext[:].rearrange("c (t m) -> (c t) m", t=2)[:, h * XCH:(h + 1) * XCH])
                            nc.vector.tensor_reduce(mxP[:, h:h + 1], xh[:], axis=AxX, op=Alu.max)
                            nc.vector.tensor_reduce(mnP[:, h:h + 1], xh[:], axis=AxX, op=Alu.min)
                        mx1 = pp.tile([128, 1], F32)
                        mn1 = pp.tile([128, 1], F32)
                        nc.vector.tensor_reduce(mx1[:], mxP[:], axis=AxX, op=Alu.max)
                        nc.vector.tensor_reduce(mn1[:], mnP[:], axis=AxX, op=Alu.min)

                        # pair-combine (c,half) partitions via DRAM bounce, expand back
                        mx128 = pp.tile([128, 1], F32)
                        for src1, dst, op in ((mx1, mx128, Alu.max), (mn1, mn128, Alu.min)):
                            nc.sync.dma_start(dram_ap(scr_stat, 0, [[1, 128]]), src1[:])
                            ev = pp.tile([64, 1], F32, tag="ev")
                            od = pp.tile([64, 1], F32, tag="od")
                            nc.sync.dma_start(ev[:], dram_ap(scr_stat, 0, [[2, 64], [1, 1]]))
                            nc.sync.dma_start(od[:], dram_ap(scr_stat, 1, [[2, 64], [1, 1]]))
                            cmb = pp.tile([64, 1], F32, tag="cmb")
                            nc.vector.tensor_tensor(cmb[:], ev[:], od[:], op=op)
                            nc.sync.dma_start(dram_ap(scr_stat2, 0, [[1, 64]]), cmb[:])
                            nc.sync.dma_start(dst[:], dram_ap(scr_stat2, 0, [[1, 64], [0, 2]]))

                        dT = pp.tile([128, 1], F32)
                        nc.vector.scalar_tensor_tensor(dT[:], mx128[:], mn128[:], epsT[:],
                                                       op0=Alu.subtract, op1=Alu.add)
                        rT = pp.tile([128, 1], F32)
                        nc.vector.reciprocal(rT[:], dT[:])
                        nc.vector.tensor_scalar(r256[:], rT[:], 256.0, None, op0=Alu.mult)

                # ============ phase 2: q (exact trunc bins -> bf16) ============
                with nc.named_scope("qcomp"):
                    with tc.tile_pool(name="xq", bufs=2) as xp2:
                        for h in range(NCH):
                            eng = nc.vector
                            sl = slice(h * XCH, (h + 1) * XCH)
                            xh = xp2.tile([128, XCH], F32, tag="xh2")
                            nc.sync.dma_start(
                                xh[:],
                                x_ext[:].rearrange("c (t m) -> (c t) m", t=2)[:, sl])
                            t_ = xp2.tile([128, XCH], F32, tag="t_")
                            eng.scalar_tensor_tensor(t_[:], xh[:], mn128[:], bcast(r256[:], xh[:]),
                                                     op0=Alu.subtract, op1=Alu.mult)
                            y_ = xp2.tile([128, XCH], F32, tag="y_")
                            eng.tensor_scalar(y_[:], t_[:], MAGIC, None, op0=Alu.add)
                            eng.tensor_scalar(y_[:], y_[:], MAGIC, None, op0=Alu.subtract)
                            g_ = xp2.tile([128, XCH], F32, tag="g_")
                            eng.tensor_tensor(g_[:], y_[:], t_[:], op=Alu.is_gt)
                            eng.tensor_tensor(y_[:], y_[:], g_[:], op=Alu.subtract)
                            eng.tensor_scalar(q_sb[:, sl], y_[:], 255.0, None, op0=Alu.min)

                # ============ phase 3: histogram (threshold counts) ============
                def emit_D(i):
                    k, h2 = divmod(i, NQI)
                    nc.vector.tensor_scalar(
                        junkD[:], q_sb[:, h2 * FQ:(h2 + 1) * FQ], float(k + 1), None,
                        op0=Alu.is_ge, op1=Alu.add,
                        accum_out=accD[:, i:i + 1])

                def emit_A(i):
                    j, h2 = divmod(i, NQI)
                    nc.scalar.activation(
                        junkA[:], q_sb[:, h2 * FQ:(h2 + 1) * FQ], Act.Sign,
                        bias=biasA[:, ND + j:ND + j + 1], scale=1.0,
                        accum_out=accA[:, i:i + 1])

                # interleave emission across engines proportionally to their
                # instruction counts so periodic cross-engine sync points land
                # at similar progress on every engine
                with nc.named_scope("hist"):
                    streams = [(emit_D, ND * NQI), (emit_A, NA * NQI)]
                    done = [0, 0]
                    total = sum(n for _, n in streams)
                    for _ in range(total):
                        si = min(range(2), key=lambda s: done[s] / streams[s][1])
                        streams[si][0](done[si])
                        done[si] += 1

            # ============ phase 4: counts + entropy ============
            with nc.named_scope("post"):
                # sum NQI sub-accumulators per threshold. DVE: pairwise tree on
                # its own accD. ACT: one activation-accum per threshold — sums
                # (0.5*x + FQ/2) over its NQI sub-accs, which IS the count
                # (each ACT sub-acc is a sum of +-1 over FQ elements). Keeping
                # each engine's reduction on that engine avoids cross-engine
                # head-of-line blocking in the in-order streams.
                def tree_sum(eng, acc_tile, width, bufs):
                    cur_tile, n_sub = acc_tile, NQI
                    while n_sub > 1:
                        half = n_sub // 2
                        nxt = bufs[half]
                        nv = nxt[:].rearrange("p (k h) -> p k h", h=half)
                        cur = cur_tile[:].rearrange("p (k h) -> p k h", h=n_sub)
                        for i2 in range(half):
                            eng.tensor_tensor(nv[:, :, i2:i2 + 1],
                                              cur[:, :, 2 * i2:2 * i2 + 1],
                                              cur[:, :, 2 * i2 + 1:2 * i2 + 2], op=Alu.add)
                        cur_tile, n_sub = nxt, half
                    return cur_tile
                sD = tree_sum(nc.vector, accD, ND, tsD_bufs)
                nc.vector.tensor_copy(cge[:, 0:ND], sD[:])
                for j in range(NA):
                    nc.scalar.activation(
                        junkS[:], accA[:, NQI * j:NQI * (j + 1)], Act.Copy,
                        bias=float(FQ // 2), scale=0.5,
                        accum_out=cge[:, ND + j:ND + j + 1])
                # combine (c, half) partition pairs -> [64, 255]
                nc.sync.dma_start(scr_cge[:], cge[:])
                cgeE = pp.tile([64, 255], F32)
                cgeO = pp.tile([64, 255], F32)
                nc.sync.dma_start(cgeE[:], dram_ap(scr_cge, 0, [[510, 64], [1, 255]]))
                nc.sync.dma_start(cgeO[:], dram_ap(scr_cge, 255, [[510, 64], [1, 255]]))
                cgeC = pp.tile([64, 255], F32)
                nc.vector.tensor_tensor(cgeC[:], cgeE[:], cgeO[:], op=Alu.add)

                nT = pp.tile([64, NBINS], F32)
                nc.vector.tensor_scalar(nT[:, 0:1], cgeC[:, 0:1], -1.0, float(N), op0=Alu.mult, op1=Alu.add)
                nc.vector.tensor_tensor(nT[:, 1:255], cgeC[:, 0:254], cgeC[:, 1:255], op=Alu.subtract)
                nc.vector.tensor_copy(nT[:, 255:256], cgeC[:, 254:255])
                nc.sync.dma_start(dbg_n[:], nT[:])

                histT = pp.tile([64, NBINS], F32)
                nc.vector.tensor_scalar(histT[:], nT[:], 1e-8, None, op0=Alu.add)
                S_ = pp.tile([64, 1], F32)
                nc.vector.tensor_reduce(S_[:], histT[:], axis=AxX, op=Alu.add)
                rS = pp.tile([64, 1], F32)
                nc.vector.reciprocal(rS[:], S_[:])
                probT = pp.tile([64, NBINS], F32)
                nc.vector.tensor_scalar(probT[:], histT[:], rS[:], None, op0=Alu.mult)
                pe_ = pp.tile([64, NBINS], F32)
                nc.vector.tensor_scalar(pe_[:], probT[:], 1e-8, None, op0=Alu.add)
                lnT = pp.tile([64, NBINS], F32)
                nc.scalar.activation(lnT[:], pe_[:], Act.Ln, bias=zb[0:64, :], scale=1.0)
                termT = pp.tile([64, NBINS], F32)
                nc.vector.tensor_tensor(termT[:], probT[:], lnT[:], op=Alu.mult)
                actT = pp.tile([64, 1], F32)
                nc.vector.tensor_reduce(actT[:], termT[:], axis=AxX, op=Alu.add, negate=True)
                nc.sync.dma_start(dbg_act[:], actT[:])

                # ============ phase 5: top-16 ============
                nc.sync.dma_start(dram_ap(scr_act, 0, [[1, 64]]), actT[:])
                act64 = pp.tile([1, 64], F32)
                nc.sync.dma_start(act64[:], dram_ap(scr_act, 0, [[64, 1], [1, 64]]))
                idx16 = pp.tile([1, 16], U32)
                m8 = pp.tile([1, 8], F32)
                nc.vector.max(m8[:], act64[:])
                nc.vector.max_index(idx16[:, 0:8], m8[:], act64[:])
                act64b = pp.tile([1, 64], F32)
                nc.vector.match_replace(act64b[:], m8[:], act64[:], -3.0e38)
                m8b = pp.tile([1, 8], F32)
                nc.vector.max(m8b[:], act64b[:])
                nc.vector.max_index(idx16[:, 8:16], m8b[:], act64b[:])
                act64c = pp.tile([1, 64], F32)
                nc.vector.match_replace(act64c[:], m8b[:], act64b[:], -3.0e38)
                nc.sync.dma_start(dbg_idx[:], idx16[:])

                # ============ phase 6: selection metadata ============
                idx16f = pp.tile([1, 16], F32)
                nc.vector.tensor_copy(idx16f[:], idx16[:])
                nc.sync.dma_start(dram_ap(scr_idx, 0, [[1, 16]]), idx16f[:])
                idx16T = pp.tile([16, 1], F32)
                nc.sync.dma_start(idx16T[:], dram_ap(scr_idx, 0, [[1, 16], [16, 1]]))
                goffF8 = pp.tile([16, 16], F32)
                for j in range(16):
                    nc.vector.tensor_scalar(goffF8[:, j:j + 1], idx16T[:], 16.0, float(j),
                                            op0=Alu.mult, op1=Alu.add)
                goffI8 = pp.tile([16, 16], I32)
                nc.vector.tensor_copy(goffI8[:], goffF8[:])

                mask01 = pp.tile([1, 64], F32)
                nc.vector.tensor_scalar(mask01[:], act64c[:], -1.0e38, None, op0=Alu.is_le)
                rank = pp.tile([1, 64], F32)
                nc.vector.tensor_tensor_scan(rank[:], mask01[:], z64[:], 0.0, op0=Alu.add, op1=Alu.add)
                sl1 = pp.tile([1, 64], F32)
                nc.vector.tensor_tensor(sl1[:], colio[:], rank[:], op=Alu.subtract)
                sl2 = pp.tile([1, 64], F32)
                nc.vector.tensor_scalar(sl2[:], sl1[:], 64.0, None, op0=Alu.add)
                slotf = pp.tile([1, 64], F32)
                nc.vector.scalar_tensor_tensor(slotf[:], mask01[:], 100000.0, sl2[:],
                                               op0=Alu.mult, op1=Alu.add)
                slot8 = pp.tile([1, 64], F32)
                nc.vector.tensor_scalar(slot8[:], slotf[:], 16.0, None, op0=Alu.mult)
                nc.sync.dma_start(dram_ap(scr_slot, 0, [[1, 64]]), slot8[:])
                slot8T = pp.tile([64, 1], F32)
                nc.sync.dma_start(slot8T[:], dram_ap(scr_slot, 0, [[1, 64], [64, 1]]))
                offsF = pp.tile([64, 16], F32)
                for j in range(16):
                    nc.vector.tensor_scalar(offsF[:, j:j + 1], slot8T[:], float(j), None, op0=Alu.add)
                offsI = pp.tile([64, 16], I32)
                nc.vector.tensor_copy(offsI[:], offsF[:])

            # ============ phase 7: selected gather + conv ============
            with nc.named_scope("conv"):
                with tc.tile_pool(name="convp", bufs=1) as cp:
                    selb = cp.tile([16, N], BF16)
                    GCH = N // 16
                    with tc.tile_pool(name="gathp", bufs=1) as gp:
                        for j in range(16):
                            ga = gp.tile([16, GCH], F32, tag="ga")
                            nc.gpsimd.indirect_dma_start(
                                out=ga[:], out_offset=None,
                                in_=x_ext[:].rearrange("c (t m) -> (c t) m", t=16),
                                in_offset=bass.IndirectOffsetOnAxis(ap=goffI8[:, j:j + 1], axis=0))
                            nc.vector.tensor_copy(selb[:, j * GCH:(j + 1) * GCH], ga[:])

                    with tc.tile_pool(name="psum", bufs=4, space="PSUM") as psp, \
                         tc.tile_pool(name="stage", bufs=2) as stp:
                        RPS = 8
                        for blk in range(32):
                            stage = stp.tile([OC, RPS * W], F32, tag="stage")
                            for yy in range(RPS):
                                y = blk * RPS + yy
                                ps = psp.tile([OC, W], F32, tag="ps")
                                taps = []
                                for dy in (-1, 0, 1):
                                    ys = y + dy
                                    if 0 <= ys < H:
                                        for dx in (-1, 0, 1):
                                            taps.append((dy, dx, ys))
                                # full-width (dx==0) tap first: the start=True
                                # matmul must zero the whole PSUM region
                                taps.sort(key=lambda t: t[1] != 0)
                                for ti, (dy, dx, ys) in enumerate(taps):
                                    t_idx = (dy + 1) * 3 + (dx + 1)
                                    if dx == -1:
                                        rhs = selb[:, ys * W:ys * W + (W - 1)]
                                        outp = ps[:, 1:W]
                                    elif dx == 1:
                                        rhs = selb[:, ys * W + 1:ys * W + W]
                                        outp = ps[:, 0:W - 1]
                                    else:
                                        rhs = selb[:, ys * W:ys * W + W]
                                        outp = ps[:, 0:W]
                                    nc.tensor.matmul(outp, wsb[:, t_idx * OC:(t_idx + 1) * OC], rhs,
                                                     start=(ti == 0), stop=(ti == len(taps) - 1))
                                nc.scalar.activation(stage[:, yy * W:(yy + 1) * W], ps[:],
                                                     Act.Identity, bias=biasC[:], scale=1.0)
                            nc.sync.dma_start(
                                out_ext[0:OC, blk * RPS * W:(blk + 1) * RPS * W], stage[:])

            # ============ phase 8: unselected passthrough ============
            # invert the channel->row map via a tiny indirect scatter of
            # channel ids into row order, then gather the 48 unselected
            # channels into SBUF and write the output tail with direct
            # contiguous DMAs (no large indirect scatter).
            with nc.named_scope("upass"):
                with tc.tile_pool(name="upmeta", bufs=1) as upm:
                    rowF = upm.tile([64, 1], F32)
                    nc.vector.tensor_scalar(rowF[:], slot8T[:], 1.0 / 16.0, None, op0=Alu.mult)
                    rowI = upm.tile([64, 1], I32)
                    nc.vector.tensor_copy(rowI[:], rowF[:])
                    cidT = upm.tile([64, 1], F32)
                    nc.sync.dma_start(cidT[:], cid_ext[:])
                    nc.gpsimd.indirect_dma_start(
                        out=dram_ap(scr_unsel, 0, [[1, 128], [1, 1]]),
                        out_offset=bass.IndirectOffsetOnAxis(ap=rowI[:], axis=0),
                        in_=cidT[:], in_offset=None,
                        bounds_check=127, oob_is_err=False)
                    unsel48 = upm.tile([48, 1], F32)
                    nc.sync.dma_start(unsel48[:], dram_ap(scr_unsel, 64, [[1, 48], [1, 1]]))
                    offs48F = upm.tile([48, 16], F32)
                    for j in range(16):
                        nc.vector.tensor_scalar(offs48F[:, j:j + 1], unsel48[:], 16.0, float(j),
                                                op0=Alu.mult, op1=Alu.add)
                    offs48I = upm.tile([48, 16], I32)
                    nc.vector.tensor_copy(offs48I[:], offs48F[:])
                    UCH = N // 16
                    with tc.tile_pool(name="upass", bufs=3) as up:
                        for j in range(16):
                            g = up.tile([48, UCH], F32, tag="ug")
                            nc.gpsimd.indirect_dma_start(
                                out=g[:], out_offset=None,
                                in_=x_ext[:].rearrange("c (t m) -> (c t) m", t=16),
                                in_offset=bass.IndirectOffsetOnAxis(ap=offs48I[:, j:j + 1], axis=0))
                            nc.sync.dma_start(out_ext[64:C_OUT, j * UCH:(j + 1) * UCH], g[:])
    nc.compile()
    return nc


_CACHED = {}


def _get_nc():
    if "nc" not in _CACHED:
        _CACHED["nc"] = build()
    return _CACHED["nc"]


def make_inputs_per_core(x, weight, bias):
    x = np.ascontiguousarray(x, dtype=np.float32)
    weight = np.asarray(weight, dtype=np.float32)
    bias = np.asarray(bias, dtype=np.float32)
    wt = np.ascontiguousarray(np.transpose(weight, (1, 2, 3, 0)).reshape(16, 9 * OC))
    biasT = np.ascontiguousarray(bias.reshape(OC, 1))
    biasA = np.ascontiguousarray(
        np.broadcast_to(-(np.arange(255, dtype=np.float32) + 0.5), (128, 255)))
    blkvec = np.ascontiguousarray((np.arange(128, dtype=np.float32) // 16).reshape(128, 1))
    colio = np.ascontiguousarray(np.arange(64, dtype=np.float32).reshape(1, 64))
    cid64 = np.ascontiguousarray(np.arange(64, dtype=np.float32).reshape(64, 1))
    maps = []
    for b in range(B):
        maps.append({
            "x": np.ascontiguousarray(x[b].reshape(C, N)),
            "w": wt, "bias": biasT, "biasA": biasA,
            "blkvec": blkvec, "colio": colio, "cid64": cid64,
        })
    return maps


LAST_RESULT = {}


def kernel(x, weight, bias):
    nc = _get_nc()
    maps = make_inputs_per_core(x, weight, bias)
    trace = bool(int(os.environ.get("KERNEL_TRACE", "0")))
    if trace:
        sys.path.insert(0, os.path.dirname(os.path.abspath(__file__)))
        try:
            import profhook
            profhook.install()
        except Exception:
            trace = False
    res = bu.run_bass_kernel_spmd(nc, maps, list(range(8)), trace=trace)
    LAST_RESULT["res"] = res
    out = np.stack([res.results[i]["out"].reshape(C_OUT, H, W) for i in range(B)])
    return out


if __name__ == "__main__":
    import reference as R
    inputs = R.setup_inputs()
    out = kernel(np.asarray(inputs["x"]), np.asarray(inputs["weight"]),
                 np.asarray(inputs["bias"]))
    print("out shape:", out.shape)


# revision 34
# speedup vs baseline: 4.4318x; 2.9398x over previous
"""AdaptivePConv Trainium2 kernel, stage 2: radix-16 joint histogram on PE.

Per core (sample b = core index):
  1. per-channel min/max -> exact bins q = floor((x-mn)*recip(mx-mn+eps)*256)
     clipped to 255 (magic-number round + fixup -> exact floor), computed in
     fp32 channel-major, then PE-transposed into pixel-major qT (fp16).
  2. histogram: q = 16h + l. One-hot planes H (16 j-planes) and L (16
     i-planes) in bf16 per group of 8 virtual channels; PE matmuls
     H^T @ L accumulate the 16x16 joint histogram of every vc into PSUM:
     counts[vc][16j+i] = sum_px H[px,(j,vc)] * L[px,(i,vc)] (block diagonal
     of the [128,128] products). 255-threshold scanning is replaced by
     ~32 one-hot passes + free PE accumulation.
  3. counts extracted via a DRAM bounce (diagonal gather), halves combined,
     entropy per channel (ACT Ln), top-16 via max8/match_replace.
  4. selected channels -> 3x3 conv as 9-tap K=16 matmuls on PE.
  5. unselected channels: rank inversion via tiny indirect scatter, then
     indirect gather + direct contiguous writes of the output tail.
"""
import os
import sys
import numpy as np

sys.path.insert(0, "/opt/trn_rl_repo")

import concourse.bass as bass
import concourse.bacc as bacc
import concourse.tile as tile
from concourse import mybir
import concourse.bass_utils as bu

F32 = mybir.dt.float32
F16 = mybir.dt.float16
BF16 = mybir.dt.bfloat16
I32 = mybir.dt.int32
U32 = mybir.dt.uint32
Alu = mybir.AluOpType
Act = mybir.ActivationFunctionType
AxX = mybir.AxisListType.X

B, C, H, W = 8, 64, 256, 256
N = H * W                  # 65536 pixels per channel
NHALF = N // 2             # 32768 per (channel, half) partition
OC, P_SEL = 64, 16
C_OUT = OC + C - P_SEL     # 112
NBINS = 256
MAGIC = float(np.float32(2.0 ** 23))
XCH = 1024                 # prepass chunk
NCH = NHALF // XCH         # 32 chunks
KC = 16                    # transpose tiles (128 px each) per one-hot batch
NT = NHALF // 128          # 256 transpose tiles
NBATCH = NT // KC          # 16


def bcast(ap_small, ap_big):
    return bass.broadcast_tensor_aps(ap_small, ap_big)[0]


def rep_ap(src_ap, pos, count):
    """Insert a zero-stride (broadcast) dim of `count` at free position pos."""
    ap = [list(d) for d in src_ap.ap]
    ap.insert(pos, [0, count])
    return bass.AP(src_ap.tensor, src_ap.offset, ap)


def build():
    nc = bacc.Bacc()
    x_ext = nc.declare_dram_parameter("x", [C, N], F32, isOutput=False)
    w_ext = nc.declare_dram_parameter("w", [16, 9 * OC], F32, isOutput=False)
    bias_ext = nc.declare_dram_parameter("bias", [OC, 1], F32, isOutput=False)
    iota_ext = nc.declare_dram_parameter("colio", [1, 64], F32, isOutput=False)
    cid_ext = nc.declare_dram_parameter("cid64", [64, 1], F32, isOutput=False)
    ident_ext = nc.declare_dram_parameter("ident", [128, 128], F32, isOutput=False)
    vmask_ext = nc.declare_dram_parameter("vmask", [128, 8], mybir.dt.uint8, isOutput=False)
    iotaJ_ext = nc.declare_dram_parameter("iotaJ", [128, 128], F32, isOutput=False)
    iotaI_ext = nc.declare_dram_parameter("iotaI", [128, 128], F32, isOutput=False)
    out_ext = nc.declare_dram_parameter("out", [C_OUT, N], F32, isOutput=True)
    dbg_act = nc.declare_dram_parameter("dbg_act", [C, 1], F32, isOutput=True)
    dbg_idx = nc.declare_dram_parameter("dbg_idx", [1, 16], U32, isOutput=True)
    dbg_n = nc.declare_dram_parameter("dbg_n", [C, NBINS], F32, isOutput=True)

    scr_stat = nc.dram_tensor("scr_stat", [128], F32)
    scr_stat2 = nc.dram_tensor("scr_stat2", [64], F32)
    scr_cnt = nc.dram_tensor("scr_cnt", [128 * 2048], F32)
    scr_c2 = nc.dram_tensor("scr_c2", [128 * 256], F32)
    scr_act = nc.dram_tensor("scr_act", [64], F32)
    scr_idx = nc.dram_tensor("scr_idx", [16], F32)
    scr_slot = nc.dram_tensor("scr_slot", [64], F32)
    scr_unsel = nc.dram_tensor("scr_unsel", [128], F32)

    def dram_ap(t, offset, pattern):
        return bass.AP(t, offset, pattern)

    with tile.TileContext(nc) as tc:
        with tc.tile_pool(name="persist", bufs=1) as pp:
            # ---- persistent small tiles ----
            wsb = pp.tile([16, 9 * OC], BF16)
            identT = pp.tile([128, 128], F32)
            iotaJh = pp.tile([128, 128], F16)
            iotaIh = pp.tile([128, 128], F16)
            zstat = pp.tile([128, 128], BF16)
            nc.vector.memset(zstat[:], 0.0)
            zmov = pp.tile([128, 512], BF16)
            nc.vector.memset(zmov[:], 0.0)
            with tc.tile_pool(name="wtmp", bufs=1) as wt:
                wsbf = wt.tile([16, 9 * OC], F32)
                nc.sync.dma_start(wsbf[:], w_ext[:])
                nc.vector.tensor_copy(wsb[:], wsbf[:])
                i32t = wt.tile([128, 128], F32, tag="i32t")
                nc.sync.dma_start(identT[:], ident_ext[:])
                nc.sync.dma_start(i32t[:], iotaJ_ext[:])
                nc.vector.tensor_copy(iotaJh[:], i32t[:])
                i32t2 = wt.tile([128, 128], F32, tag="i32t2")
                nc.sync.dma_start(i32t2[:], iotaI_ext[:])
                nc.vector.tensor_copy(iotaIh[:], i32t2[:])
            biasC = pp.tile([OC, 1], F32)
            nc.sync.dma_start(biasC[:], bias_ext[:])
            vmask = pp.tile([128, 8], mybir.dt.uint8)
            nc.sync.dma_start(vmask[:], vmask_ext[:])
            colio = pp.tile([1, 64], F32)
            nc.sync.dma_start(colio[:], iota_ext[:])
            epsT = pp.tile([128, 1], F32)
            nc.vector.memset(epsT[:], 1e-8)
            zb = pp.tile([128, 1], F32)
            nc.vector.memset(zb[:], 0.0)
            z64 = pp.tile([1, 64], F32)
            nc.vector.memset(z64[:], 0.0)

            mn128 = pp.tile([128, 1], F32)
            r256 = pp.tile([128, 1], F32)
            # histogram result tiles live in their own pool, held open from
            # before the phase-2/3 pools until the end of post, so the
            # allocator cannot overlap them with scoped-pool tiles
            postp_ctx = tc.tile_pool(name="postp", bufs=1)
            postp = postp_ctx.__enter__()
            pg = postp.tile([128, 2048], F32)
            dcomp = postp.tile([128, NBINS], F32)
            counts = postp.tile([128, NBINS], F32)
            nT = pp.tile([64, NBINS], F32)

            # ============ phase 1: min/max ============
            with nc.named_scope("minmax"):
                with tc.tile_pool(name="xminmax", bufs=3) as xp:
                    mxP = pp.tile([128, NCH], F32)
                    mnP = pp.tile([128, NCH], F32)
                    for h in range(NCH):
                        xh = xp.tile([128, XCH], F32, tag="xh")
                        nc.sync.dma_start(
                            xh[:],
                            x_ext[:].rearrange("c (t m) -> (c t) m", t=2)[:, h * XCH:(h + 1) * XCH])
                        nc.vector.tensor_reduce(mxP[:, h:h + 1], xh[:], axis=AxX, op=Alu.max)
                        nc.vector.tensor_reduce(mnP[:, h:h + 1], xh[:], axis=AxX, op=Alu.min)
                    mx1 = pp.tile([128, 1], F32)
                    mn1 = pp.tile([128, 1], F32)
                    nc.vector.tensor_reduce(mx1[:], mxP[:], axis=AxX, op=Alu.max)
                    nc.vector.tensor_reduce(mn1[:], mnP[:], axis=AxX, op=Alu.min)

                    # pair-combine (c,half) partitions via DRAM bounce, expand back
                    mx128 = pp.tile([128, 1], F32)
                    for src1, dst, op in ((mx1, mx128, Alu.max), (mn1, mn128, Alu.min)):
                        nc.sync.dma_start(dram_ap(scr_stat, 0, [[1, 128]]), src1[:])
                        ev = pp.tile([64, 1], F32, tag="ev")
                        od = pp.tile([64, 1], F32, tag="od")
                        nc.sync.dma_start(ev[:], dram_ap(scr_stat, 0, [[2, 64], [1, 1]]))
                        nc.sync.dma_start(od[:], dram_ap(scr_stat, 1, [[2, 64], [1, 1]]))
                        cmb = pp.tile([64, 1], F32, tag="cmb")
                        nc.vector.tensor_tensor(cmb[:], ev[:], od[:], op=op)
                        nc.sync.dma_start(dram_ap(scr_stat2, 0, [[1, 64]]), cmb[:])
                        nc.sync.dma_start(dst[:], dram_ap(scr_stat2, 0, [[1, 64], [0, 2]]))

                    dT = pp.tile([128, 1], F32)
                    nc.vector.scalar_tensor_tensor(dT[:], mx128[:], mn128[:], epsT[:],
                                                   op0=Alu.subtract, op1=Alu.add)
                    rT = pp.tile([128, 1], F32)
                    nc.vector.reciprocal(rT[:], dT[:])
                    nc.vector.tensor_scalar(r256[:], rT[:], 256.0, None, op0=Alu.mult)

            # ====== phase 2: q = exact floor bins, fused PE transpose -> qT ======
            qtp_ctx = tc.tile_pool(name="qtp", bufs=1)
            qtp = qtp_ctx.__enter__()
            qT = qtp.tile([128, NHALF], F16)
            with nc.named_scope("qcomp"):
                with tc.tile_pool(name="xq", bufs=2) as xp2, \
                     tc.tile_pool(name="tpsum", bufs=2, space="PSUM") as tps:
                    for h in range(NCH):
                        sl = slice(h * XCH, (h + 1) * XCH)
                        xh = xp2.tile([128, XCH], F32, tag="xh2")
                        nc.sync.dma_start(
                            xh[:],
                            x_ext[:].rearrange("c (t m) -> (c t) m", t=2)[:, sl])
                        t_ = xp2.tile([128, XCH], F32, tag="t_")
                        nc.vector.scalar_tensor_tensor(t_[:], xh[:], mn128[:], bcast(r256[:], xh[:]),
                                                       op0=Alu.subtract, op1=Alu.mult)
                        y_ = xp2.tile([128, XCH], F32, tag="y_")
                        nc.vector.tensor_scalar(y_[:], t_[:], MAGIC, None, op0=Alu.add)
                        nc.vector.tensor_scalar(y_[:], y_[:], MAGIC, None, op0=Alu.subtract)
                        g_ = xp2.tile([128, XCH], F32, tag="g_")
                        nc.vector.tensor_tensor(g_[:], y_[:], t_[:], op=Alu.is_gt)
                        nc.vector.tensor_tensor(y_[:], y_[:], g_[:], op=Alu.subtract)
                        yq = xp2.tile([128, XCH], F32, tag="yq")
                        nc.vector.tensor_scalar(yq[:], y_[:], 255.0, None, op0=Alu.min)
                        for s2 in range(2):
                            pst = tps.tile([128, 512], F32, tag="pst")
                            for s in range(4):
                                col = s2 * 512 + s * 128
                                nc.tensor.transpose(pst[:, s * 128:(s + 1) * 128],
                                                    yq[:, col:col + 128], identT[:])
                            nc.scalar.activation(qT[:, h * XCH + s2 * 512:h * XCH + s2 * 512 + 512],
                                                 pst[:], Act.Copy, bias=0.0, scale=1.0)

            # ============ phase 3: one-hot + PE joint histogram ============
            with nc.named_scope("hist"):
                with tc.tile_pool(name="hl", bufs=2) as hlp, \
                     tc.tile_pool(name="oh", bufs=3) as ohp, \
                     tc.tile_pool(name="hpsum", bufs=1, space="PSUM") as hps:
                    psB = [hps.tile([128, 512], F32, name=f"psB{i}", tag=f"psB{i}")
                           for i in range(4)]
                    # zero each bank once (bank-wide), then accumulate-only:
                    # per-region start=True would re-zero the whole bank and
                    # wipe sibling groups' accumulations
                    for i in range(4):
                        nc.tensor.matmul(psB[i][:], zstat[:], zmov[:],
                                         start=True, stop=False, skip_group_check=True)
                    for b in range(NBATCH):
                        qb = qT[:, b * KC * 128:(b + 1) * KC * 128]
                        t2 = hlp.tile([128, KC * 128], F16, tag="t2")
                        nc.scalar.activation(t2[:], qb, Act.Copy, bias=0.0, scale=1.0 / 16.0)
                        y2 = hlp.tile([128, KC * 128], F16, tag="y2")
                        nc.scalar.activation(y2[:], qb, Act.Copy, bias=1024.0, scale=1.0 / 16.0)
                        hb0 = hlp.tile([128, KC * 128], F16, tag="hb0")
                        nc.vector.tensor_scalar(hb0[:], y2[:], -1024.0, None, op0=Alu.add)
                        g2 = hlp.tile([128, KC * 128], F16, tag="g2")
                        nc.vector.tensor_tensor(g2[:], hb0[:], t2[:], op=Alu.is_gt)
                        hb = hlp.tile([128, KC * 128], F16, tag="hb")
                        nc.vector.tensor_tensor(hb[:], hb0[:], g2[:], op=Alu.subtract)
                        lb = hlp.tile([128, KC * 128], F16, tag="lb")
                        nc.vector.scalar_tensor_tensor(lb[:], hb[:], -16.0, qb,
                                                       op0=Alu.mult, op1=Alu.add)
                        hbv = hb[:].rearrange("p (kc c) -> p kc c", c=128)
                        lbv = lb[:].rearrange("p (kc c) -> p kc c", c=128)
                        jv = iotaJh[:].rearrange("p (j v) -> p j v", v=8)
                        iv = iotaIh[:].rearrange("p (j v) -> p j v", v=8)
                        for g in range(16):
                            HB = ohp.tile([128, KC * 128], BF16, tag="HB")
                            LB = ohp.tile([128, KC * 128], BF16, tag="LB")
                            for src, iview, out_t in ((hbv, jv, HB), (lbv, iv, LB)):
                                o = out_t[:].rearrange("p (kc j v) -> p kc j v", j=16, v=8)
                                s0 = src[:, :, g * 8:(g + 1) * 8]
                                s = rep_ap(s0, 2, 16)
                                it = rep_ap(iview, 1, KC)
                                nc.vector.tensor_tensor(o, s, it, op=Alu.is_equal)
                            bank = psB[g // 4][:, (g % 4) * 128:(g % 4 + 1) * 128]
                            for c2 in range(KC):
                                nc.tensor.matmul(bank,
                                                 HB[:, c2 * 128:(c2 + 1) * 128],
                                                 LB[:, c2 * 128:(c2 + 1) * 128],
                                                 start=False,
                                                 stop=(b == NBATCH - 1 and c2 == KC - 1),
                                                 skip_group_check=True)
                    for i in range(4):
                        nc.scalar.activation(pg[:, i * 512:(i + 1) * 512], psB[i][:],
                                             Act.Copy, bias=0.0, scale=1.0)
            qtp_ctx.__exit__(None, None, None)

            # ============ phase 4: counts + entropy ============
            with nc.named_scope("post"):
                # pg row (j,v) holds cols (g, i, v'); the needed diagonal v'==v
                # is a fixed per-partition column offset (v = p%8). For each v:
                # strided-copy columns (g, i, v) for ALL partitions, then merge
                # the rows whose p%8==v via copy_predicated. Avoids
                # partition-strided APs (race detector can't reason about them).
                pgv = pg[:].rearrange("p (g i w) -> p g i w", i=16, w=8)
                dcv = dcomp[:].rearrange("p (g i) -> p g i", i=16)
                for v in range(8):
                    dsel = postp.tile([128, NBINS], F32, tag=f"dsel{v % 2}",
                                      name=f"dsel{v % 2}")
                    dsv = dsel[:].rearrange("p (g i) -> p g i", i=16)
                    nc.vector.tensor_copy(dsv, pgv[:, :, :, v:v + 1])
                    nc.vector.copy_predicated(dcomp[:], bcast(vmask[:, v:v + 1], dsel[:]),
                                              dsel[:])
                nc.sync.dma_start(dram_ap(scr_cnt, 0, [[256, 128], [1, 256]]), dcomp[:])
                for j in range(16):
                    nc.sync.dma_start(
                        counts[:, j * 16:(j + 1) * 16],
                        dram_ap(scr_cnt, j * 2048, [[16, 16], [256, 8], [1, 16]]))
                # combine (c, half) partition pairs -> [64, 256]
                nc.sync.dma_start(dram_ap(scr_c2, 0, [[256, 128], [1, 256]]), counts[:])
                cE = pp.tile([64, NBINS], F32)
                cO = pp.tile([64, NBINS], F32)
                nc.sync.dma_start(cE[:], dram_ap(scr_c2, 0, [[512, 64], [1, 256]]))
                nc.sync.dma_start(cO[:], dram_ap(scr_c2, 256, [[512, 64], [1, 256]]))
                nc.vector.tensor_tensor(nT[:], cE[:], cO[:], op=Alu.add)
                nc.sync.dma_start(dbg_n[:], nT[:])

                histT = pp.tile([64, NBINS], F32)
                nc.vector.tensor_scalar(histT[:], nT[:], 1e-8, None, op0=Alu.add)
                S_ = pp.tile([64, 1], F32)
                nc.vector.tensor_reduce(S_[:], histT[:], axis=AxX, op=Alu.add)
                rS = pp.tile([64, 1], F32)
                nc.vector.reciprocal(rS[:], S_[:])
                probT = pp.tile([64, NBINS], F32)
                nc.vector.tensor_scalar(probT[:], histT[:], rS[:], None, op0=Alu.mult)
                pe_ = pp.tile([64, NBINS], F32)
                nc.vector.tensor_scalar(pe_[:], probT[:], 1e-8, None, op0=Alu.add)
                lnT = pp.tile([64, NBINS], F32)
                nc.scalar.activation(lnT[:], pe_[:], Act.Ln, bias=zb[0:64, :], scale=1.0)
                termT = pp.tile([64, NBINS], F32)
                nc.vector.tensor_tensor(termT[:], probT[:], lnT[:], op=Alu.mult)
                actT = pp.tile([64, 1], F32)
                nc.vector.tensor_reduce(actT[:], termT[:], axis=AxX, op=Alu.add, negate=True)
                nc.sync.dma_start(dbg_act[:], actT[:])
                postp_ctx.__exit__(None, None, None)

                # ============ phase 5: top-16 ============
                nc.sync.dma_start(dram_ap(scr_act, 0, [[1, 64]]), actT[:])
                act64 = pp.tile([1, 64], F32)
                nc.sync.dma_start(act64[:], dram_ap(scr_act, 0, [[64, 1], [1, 64]]))
                idx16 = pp.tile([1, 16], U32)
                m8 = pp.tile([1, 8], F32)
                nc.vector.max(m8[:], act64[:])
                nc.vector.max_index(idx16[:, 0:8], m8[:], act64[:])
                act64b = pp.tile([1, 64], F32)
                nc.vector.match_replace(act64b[:], m8[:], act64[:], -3.0e38)
                m8b = pp.tile([1, 8], F32)
                nc.vector.max(m8b[:], act64b[:])
                nc.vector.max_index(idx16[:, 8:16], m8b[:], act64b[:])
                act64c = pp.tile([1, 64], F32)
                nc.vector.match_replace(act64c[:], m8b[:], act64b[:], -3.0e38)
                nc.sync.dma_start(dbg_idx[:], idx16[:])

                # ============ phase 6: selection metadata ============
                idx16f = pp.tile([1, 16], F32)
                nc.vector.tensor_copy(idx16f[:], idx16[:])
                nc.sync.dma_start(dram_ap(scr_idx, 0, [[1, 16]]), idx16f[:])
                idx16T = pp.tile([16, 1], F32)
                nc.sync.dma_start(idx16T[:], dram_ap(scr_idx, 0, [[1, 16], [16, 1]]))
                goffF8 = pp.tile([16, 16], F32)
                for j in range(16):
                    nc.vector.tensor_scalar(goffF8[:, j:j + 1], idx16T[:], 16.0, float(j),
                                            op0=Alu.mult, op1=Alu.add)
                goffI8 = pp.tile([16, 16], I32)
                nc.vector.tensor_copy(goffI8[:], goffF8[:])

                mask01 = pp.tile([1, 64], F32)
                nc.vector.tensor_scalar(mask01[:], act64c[:], -1.0e38, None, op0=Alu.is_le)
                rank = pp.tile([1, 64], F32)
                nc.vector.tensor_tensor_scan(rank[:], mask01[:], z64[:], 0.0, op0=Alu.add, op1=Alu.add)
                sl1 = pp.tile([1, 64], F32)
                nc.vector.tensor_tensor(sl1[:], colio[:], rank[:], op=Alu.subtract)
                sl2 = pp.tile([1, 64], F32)
                nc.vector.tensor_scalar(sl2[:], sl1[:], 64.0, None, op0=Alu.add)
                slotf = pp.tile([1, 64], F32)
                nc.vector.scalar_tensor_tensor(slotf[:], mask01[:], 100000.0, sl2[:],
                                               op0=Alu.mult, op1=Alu.add)
                slot8 = pp.tile([1, 64], F32)
                nc.vector.tensor_scalar(slot8[:], slotf[:], 16.0, None, op0=Alu.mult)
                nc.sync.dma_start(dram_ap(scr_slot, 0, [[1, 64]]), slot8[:])
                slot8T = pp.tile([64, 1], F32)
                nc.sync.dma_start(slot8T[:], dram_ap(scr_slot, 0, [[1, 64], [64, 1]]))

            # ============ phase 7: selected gather + conv ============
            with nc.named_scope("conv"):
                with tc.tile_pool(name="convp", bufs=1) as cp:
                    selb = cp.tile([16, N], BF16)
                    GCH = N // 16
                    with tc.tile_pool(name="gathp", bufs=1) as gp:
                        for j in range(16):
                            ga = gp.tile([16, GCH], F32, tag="ga")
                            nc.gpsimd.indirect_dma_start(
                                out=ga[:], out_offset=None,
                                in_=x_ext[:].rearrange("c (t m) -> (c t) m", t=16),
                                in_offset=bass.IndirectOffsetOnAxis(ap=goffI8[:, j:j + 1], axis=0))
                            nc.vector.tensor_copy(selb[:, j * GCH:(j + 1) * GCH], ga[:])

                    with tc.tile_pool(name="psum", bufs=4, space="PSUM") as psp, \
                         tc.tile_pool(name="stage", bufs=2) as stp:
                        RPS = 8
                        for blk in range(32):
                            stage = stp.tile([OC, RPS * W], F32, tag="stage")
                            for yy in range(RPS):
                                y = blk * RPS + yy
                                ps = psp.tile([OC, W], F32, tag="ps")
                                taps = []
                                for dy in (-1, 0, 1):
                                    ys = y + dy
                                    if 0 <= ys < H:
                                        for dx in (-1, 0, 1):
                                            taps.append((dy, dx, ys))
                                # full-width (dx==0) tap first: the start=True
                                # matmul must zero the whole PSUM region
                                taps.sort(key=lambda t: t[1] != 0)
                                for ti, (dy, dx, ys) in enumerate(taps):
                                    t_idx = (dy + 1) * 3 + (dx + 1)
                                    if dx == -1:
                                        rhs = selb[:, ys * W:ys * W + (W - 1)]
                                        outp = ps[:, 1:W]
                                    elif dx == 1:
                                        rhs = selb[:, ys * W + 1:ys * W + W]
                                        outp = ps[:, 0:W - 1]
                                    else:
                                        rhs = selb[:, ys * W:ys * W + W]
                                        outp = ps[:, 0:W]
                                    nc.tensor.matmul(outp, wsb[:, t_idx * OC:(t_idx + 1) * OC], rhs,
                                                     start=(ti == 0), stop=(ti == len(taps) - 1))
                                nc.scalar.activation(stage[:, yy * W:(yy + 1) * W], ps[:],
                                                     Act.Identity, bias=biasC[:], scale=1.0)
                            nc.sync.dma_start(
                                out_ext[0:OC, blk * RPS * W:(blk + 1) * RPS * W], stage[:])

            # ============ phase 8: unselected passthrough ============
            with nc.named_scope("upass"):
                with tc.tile_pool(name="upmeta", bufs=1) as upm:
                    rowF = upm.tile([64, 1], F32)
                    nc.vector.tensor_scalar(rowF[:], slot8T[:], 1.0 / 16.0, None, op0=Alu.mult)
                    rowI = upm.tile([64, 1], I32)
                    nc.vector.tensor_copy(rowI[:], rowF[:])
                    cidT = upm.tile([64, 1], F32)
                    nc.sync.dma_start(cidT[:], cid_ext[:])
                    nc.gpsimd.indirect_dma_start(
                        out=dram_ap(scr_unsel, 0, [[1, 128], [1, 1]]),
                        out_offset=bass.IndirectOffsetOnAxis(ap=rowI[:], axis=0),
                        in_=cidT[:], in_offset=None,
                        bounds_check=127, oob_is_err=False)
                    unsel48 = upm.tile([48, 1], F32)
                    nc.sync.dma_start(unsel48[:], dram_ap(scr_unsel, 64, [[1, 48], [1, 1]]))
                    offs48F = upm.tile([48, 16], F32)
                    for j in range(16):
                        nc.vector.tensor_scalar(offs48F[:, j:j + 1], unsel48[:], 16.0, float(j),
                                                op0=Alu.mult, op1=Alu.add)
                    offs48I = upm.tile([48, 16], I32)
                    nc.vector.tensor_copy(offs48I[:], offs48F[:])
                    UCH = N // 16
                    with tc.tile_pool(name="upass", bufs=3) as up:
                        for j in range(16):
                            g = up.tile([48, UCH], F32, tag="ug")
                            nc.gpsimd.indirect_dma_start(
                                out=g[:], out_offset=None,
                                in_=x_ext[:].rearrange("c (t m) -> (c t) m", t=16),
                                in_offset=bass.IndirectOffsetOnAxis(ap=offs48I[:, j:j + 1], axis=0))
                            nc.sync.dma_start(out_ext[64:C_OUT, j * UCH:(j + 1) * UCH], g[:])
    nc.compile()
    return nc


_CACHED = {}


def _get_nc():
    if "nc" not in _CACHED:
        _CACHED["nc"] = build()
    return _CACHED["nc"]


def make_inputs_per_core(x, weight, bias):
    x = np.ascontiguousarray(x, dtype=np.float32)
    weight = np.asarray(weight, dtype=np.float32)
    bias = np.asarray(bias, dtype=np.float32)
    wt = np.ascontiguousarray(np.transpose(weight, (1, 2, 3, 0)).reshape(16, 9 * OC))
    biasT = np.ascontiguousarray(bias.reshape(OC, 1))
    colio = np.ascontiguousarray(np.arange(64, dtype=np.float32).reshape(1, 64))
    cid64 = np.ascontiguousarray(np.arange(64, dtype=np.float32).reshape(64, 1))
    ident = np.eye(128, dtype=np.float32)
    cols = np.arange(128)
    iotaJ = np.ascontiguousarray(
        np.broadcast_to((cols // 8).astype(np.float32), (128, 128)))
    iotaI = iotaJ.copy()
    vmask = np.ascontiguousarray(
        (np.arange(128)[:, None] % 8 == np.arange(8)[None, :]).astype(np.uint8))
    maps = []
    for b in range(B):
        maps.append({
            "x": np.ascontiguousarray(x[b].reshape(C, N)),
            "w": wt, "bias": biasT, "colio": colio, "cid64": cid64,
            "ident": ident, "iotaJ": iotaJ, "iotaI": iotaI, "vmask": vmask,
        })
    return maps


LAST_RESULT = {}


def kernel(x, weight, bias):
    nc = _get_nc()
    maps = make_inputs_per_core(x, weight, bias)
    trace = bool(int(os.environ.get("KERNEL_TRACE", "0")))
    if trace:
        sys.path.insert(0, os.path.dirname(os.path.abspath(__file__)))
        try:
            import profhook
            profhook.install()
        except Exception:
            trace = False
    res = bu.run_bass_kernel_spmd(nc, maps, list(range(8)), trace=trace)
    LAST_RESULT["res"] = res
    out = np.stack([res.results[i]["out"].reshape(C_OUT, H, W) for i in range(B)])
    return out


if __name__ == "__main__":
    import reference as R
    inputs = R.setup_inputs()
    out = kernel(np.asarray(inputs["x"]), np.asarray(inputs["weight"]),
                 np.asarray(inputs["bias"]))
    print("out shape:", out.shape)
